# revision 40
# baseline (speedup 1.0000x reference)
"""Multi-head GQA attention (B=2, S=2048, H=4096, 32 q-heads / 8 kv-heads,
HD=128, rotary, causal) on 8 TRN2 NeuronCores.

Sharding: tensor-parallel over heads, 8-way — core c owns q-heads
[4c, 4c+4) and kv-head c; wq/wk/wv column-sharded, wo row-sharded.  Each
core computes a partial wo product over its head slice for both batches;
the host sums the 8 partials (the TP reduction) and transposes back.

All on-device dataflow is in transposed layout [feature, seq] so no
device-side transposes are needed; the host pre-transposes x and the
weight shards during sharding.  Rotary pairs are laid out so the (x0,x1)
pair swap is a 32-partition-quadrant stream_shuffle on the Vector engine.

v5 structural changes vs v2/v3:
- the attention phase is ACT-bound (exp is ~687ns/tile, not pipelined),
  so attention for block n-1 runs during block n, with the wo matmuls of
  block n-2 interleaved one 6-ho chunk after each head: the PE chews wo
  while ACT streams exps, and neither waits for the other.
- attention lagging a full block means rotary/kT/vn never gate scores
  (50us of slack), so the old half1/half2 rotary-coverage machinery is
  gone; a small 8-ho wo prefix before each projection covers the x DMA.
- softmax denominators accumulate on DVE in bf16 (tile adds); one
  ones-matmul per head at its chunk end replaces the per-tile sums
  matmul (PE pays 512 cols/head instead of a full extra sums pass).
- PSUM is re-tagged: a 4-deep rotation (qa x4 / tp / wo accs / sm),
  ka/vta borrow the two score banks (disjoint phases), and pvs get two
  dedicated banks - projection still sees 6 live banks, attention gets
  a collision-free pipeline.
- rotary runs fully on DVE in bf16 (freqs shipped bf16): ~2x cheaper
  and off ACT's critical path.
"""
import sys

if "/opt/trn_rl_repo" not in sys.path:
    sys.path.insert(0, "/opt/trn_rl_repo")

import numpy as np
import ml_dtypes

from concourse import bacc, tile, mybir
from concourse.bass_utils import run_bass_kernel_spmd

F32 = mybir.dt.float32
BF16 = mybir.dt.bfloat16
EXP = mybir.ActivationFunctionType.Exp
COPY = mybir.ActivationFunctionType.Copy
BF16NP = ml_dtypes.bfloat16

B, S, H = 2, 2048, 4096
NH, NKV, HD = 32, 8, 128
NCORES = 8
QH = NH // NCORES          # 4 q-heads per core
RQ = QH * HD               # 512 q rows per core
SB = 512                   # projection seq block
NSB = S // SB              # 4
IB = 512                   # attention i block
NIB = S // IB              # 4
HC = H // 128              # 32 contraction chunks
NJT = S // 128             # 16 j tiles
NHO = H // 128             # 32 wo output row-chunks
WO_PRE = 4                 # wo chunks emitted before the projection

# stream_shuffle mask: swap 16-partition halves within each 32-partition quadrant
SHUF = list(range(16, 32)) + list(range(16))

LAST_EXEC_NS = None
LAST_RES = None
_CACHED = None


def _build():
    nc = bacc.Bacc("TRN2", target_bir_lowering=False, debug=False,
                   num_devices=NCORES)

    xt_e = nc.dram_tensor("xt", [B, H, S], BF16, kind="ExternalInput")
    cc_e = nc.dram_tensor("cc", [128, S], BF16, kind="ExternalInput")
    ss_e = nc.dram_tensor("ss2", [128, S], BF16, kind="ExternalInput")
    wq_e = nc.dram_tensor("wqt", [H, RQ], BF16, kind="ExternalInput")
    wk_e = nc.dram_tensor("wkt", [H, HD], BF16, kind="ExternalInput")
    wv_e = nc.dram_tensor("wvt", [H, HD], BF16, kind="ExternalInput")
    wo_e = nc.dram_tensor("wot", [RQ, H], BF16, kind="ExternalInput")
    mk_e = nc.dram_tensor("maskd", [128, 128], BF16, kind="ExternalInput")
    mk4_e = nc.dram_tensor("maskd4", [128, 512], BF16, kind="ExternalInput")
    id_e = nc.dram_tensor("ident", [128, 128], BF16, kind="ExternalInput")
    out_e = nc.dram_tensor("out", [B, H, S], BF16, kind="ExternalOutput")

    with tile.TileContext(nc) as tc:
        with (nc.allow_low_precision(reason="bf16 compute by design"),
              tc.tile_pool(name="wpool", bufs=1) as wp,
              tc.tile_pool(name="state", bufs=1) as st,
              tc.tile_pool(name="att", bufs=3) as ap_,
              tc.tile_pool(name="xin", bufs=4) as xp,
              tc.tile_pool(name="probs", bufs=1) as pp,
              tc.tile_pool(name="rot", bufs=1) as rp,
              tc.tile_pool(name="stage", bufs=1) as sg,
              tc.tile_pool(name="ps", bufs=4, space="PSUM") as ps):

            # ---- resident weights ----
            wq_s = wp.tile([128, HC, RQ], BF16)
            wk_s = wp.tile([128, HC, HD], BF16)
            wv_s = wp.tile([128, HC, HD], BF16)
            wo_s = wp.tile([128, QH, H], BF16)
            mk_s = wp.tile([128, 128], BF16)
            mk4_s = wp.tile([128, QH, 128], BF16)
            ones128 = wp.tile([128, 128], BF16)
            id_s = wp.tile([128, 128], BF16)
            nc.vector.memset(ones128[:], 1.0)

            # PE warm-up: harmless matmuls that run during the initial DMA
            # wait; they pull the PE out of its low p-state and cover the
            # first x/weight transfers
            warm = wp.tile([128, 512], BF16)
            nc.vector.memset(warm[:], 0.0)
            wps = ps.tile([128, 512], F32, tag="ps", name="wps")
            for i in range(12):
                nc.tensor.matmul(wps[:], ones128[:], warm[:],
                                 start=(i == 0), stop=(i == 11))

            _wchunk_loaded = [False] * (HC // 4)
            _tail_loaded = [False]
            _wo_loaded = [False]

            def load_wchunk(hg):
                # one grouped DMA per 4 contraction chunks.  wq alternates
                # sync/gpsimd, wk+wv ride the otherwise-idle scalar queue, and
                # xt uses the opposite parity: each queue then delivers in
                # consumption order and none serializes the startup.
                if _wchunk_loaded[hg]:
                    return
                _wchunk_loaded[hg] = True
                eng = nc.sync if hg % 2 == 0 else nc.gpsimd
                r0, r1 = hg * 512, (hg + 1) * 512
                eng.dma_start(
                    out=wq_s[:, 4 * hg:4 * hg + 4],
                    in_=wq_e.ap()[r0:r1].rearrange("(c p) r -> p c r", c=4))
                nc.scalar.dma_start(
                    out=wk_s[:, 4 * hg:4 * hg + 4],
                    in_=wk_e.ap()[r0:r1].rearrange("(c p) r -> p c r", c=4))
                nc.scalar.dma_start(
                    out=wv_s[:, 4 * hg:4 * hg + 4],
                    in_=wv_e.ap()[r0:r1].rearrange("(c p) r -> p c r", c=4))

            def load_tail_weights():
                if _tail_loaded[0]:
                    return
                _tail_loaded[0] = True
                nc.gpsimd.dma_start(out=id_s[:], in_=id_e.ap())
                nc.gpsimd.dma_start(out=mk_s[:], in_=mk_e.ap())
                nc.gpsimd.dma_start(out=mk4_s[:], in_=mk4_e.ap())

            def load_wo():
                # wo (4MB) is first read during block 2; loading it in block
                # 0 oversubscribes the startup DMA window, so it goes out on
                # the idle scalar queue at block 1
                if _wo_loaded[0]:
                    return
                _wo_loaded[0] = True
                for rc in range(QH):
                    nc.scalar.dma_start(out=wo_s[:, rc],
                                        in_=wo_e.ap()[rc * 128:(rc + 1) * 128])

            # ---- wo emission: one at-tile is consumed as 32 ho chunks ----
            _og_n = [0]

            def emit_wo(wb, wib, wat, ho_lo, ho_hi, og_eng, final=False):
                wi0 = wib * IB
                for ho in range(ho_lo, ho_hi):
                    acc = ps.tile([128, IB], F32, tag="ps", name="acc")
                    for rc in range(QH):
                        nc.tensor.matmul(acc[:],
                                         wo_s[:, rc, ho * 128:(ho + 1) * 128],
                                         wat[:, rc],
                                         start=(rc == 0), stop=(rc == QH - 1))
                    g = _og_n[0] = (_og_n[0] + 1) % 8
                    og = sg.tile([128, IB], BF16, tag=f"og{g}", name="og",
                                 bufs=1)
                    if og_eng == "act" or (og_eng == "mix" and ho % 2 == 0):
                        nc.scalar.activation(og[:], acc[:], COPY)
                    else:
                        nc.vector.tensor_copy(og[:], acc[:])
                    if final:
                        oeng = (nc.gpsimd, nc.sync, nc.scalar)[ho % 3]
                    else:
                        oeng = nc.gpsimd if ho % 2 == 0 else nc.sync
                    oeng.dma_start(
                        out=out_e.ap()[wb, ho * 128:(ho + 1) * 128, wi0:wi0 + IB],
                        in_=og[:])

            # freqs are identical for both batches: load once (bf16)
            ccb = st.tile([128, S], BF16, tag="cc")
            ssb = st.tile([128, S], BF16, tag="ss")
            _freqs_loaded = [False]

            _rot_n = [0]

            def rot_stage(src_ps, tag, cp="dve"):
                # PSUM -> SBUF bf16 staging.  Staging all five rotary inputs
                # FIRST releases their PSUM banks quickly: the v-transpose
                # reuses qa0's slot and the next attention's first scores
                # matmul reuses ka's (sc0), so these reads gate the PE.
                qs = rp.tile([128, SB], BF16, tag=tag, bufs=1)
                if cp == "act":
                    nc.scalar.activation(qs[:], src_ps[:], COPY)
                else:
                    nc.vector.tensor_copy(qs[:], src_ps[:])
                return qs

            def rot_math(qs, s0, dst_ap):
                # dst = qs*CC + shuffle(qs)*SS2, bf16 on DVE
                r = _rot_n[0] = (_rot_n[0] + 1) % 2
                qw = rp.tile([128, SB], BF16, tag=f"qw{r}", bufs=1)
                nc.vector.stream_shuffle(qw[:], qs[:], SHUF)
                nc.vector.tensor_mul(qs[:], qs[:], ccb[:, s0:s0 + SB])
                nc.vector.tensor_mul(qw[:], qw[:], ssb[:, s0:s0 + SB])
                nc.vector.tensor_add(dst_ap, qs[:], qw[:])

            # ---- attention for one (b, ib), interleaved with wo chunks ----
            def attention(b, ib, qT, kT, vn, wo_args):
                # wo_args: (wb, wib, wat) of the 2-blocks-ago at, or None.
                # Emits a 6-ho wo chunk after each head so the PE stays busy
                # while ACT streams the next head's exps; the chunk also
                # gives the DVE time to finish the head's prob-sum adds
                # before the ones-matmul (finalize) needs them.
                i0 = ib * IB
                njt = (ib + 1) * (IB // 128)
                at = ap_.tile([128, QH, IB], BF16, tag="at", name="at")
                pvs = [None] * QH
                sms = [None] * QH
                accs = [None] * QH
                # without wo filler (block 1 only) the chunk trick can't
                # hide the DVE lag: keep per-tile PE sums there
                dve_sums = wo_args is not None
                pend = []
                dstate = {}    # merged diagonal-tile probs, shared by heads

                def flush():
                    ph, pjt, ppb, pf0 = pend.pop(0)
                    st_ = (pjt == 0)
                    sp_ = (pjt == njt - 1)
                    nc.tensor.matmul(pvs[ph][:, pf0:], vn[:, pjt], ppb,
                                     start=st_, stop=sp_)
                    if not dve_sums:
                        nc.tensor.matmul(sms[ph][:, pf0:], ones128[:], ppb,
                                         start=st_, stop=sp_)
                        if sp_:
                            rb = sg.tile([128, IB], F32, tag=f"rb{ph % 2}",
                                         bufs=1, name="rb")
                            nc.vector.reciprocal_approx_fast(rb[:], sms[ph][:])
                            nc.vector.tensor_mul(at[:, ph], pvs[ph][:], rb[:])

                def finalize(ph):
                    sm = ps.tile([128, IB], F32, tag="ps", name=f"sm{ph}")
                    nc.tensor.matmul(sm[:], ones128[:], accs[ph][:],
                                     start=True, stop=True)
                    rb = sg.tile([128, IB], F32, tag=f"rb{ph % 2}",
                                 bufs=1, name="rb")
                    nc.vector.reciprocal_approx_fast(rb[:], sm[:])
                    nc.vector.tensor_mul(at[:, ph], pvs[ph][:], rb[:])

                # wo chunk schedule: 6 ho per head when in the steady loop
                # (prefix of 8 emitted before the projection), 8 per head in
                # the tail where there is no prefix
                if wo_args is not None:
                    per_head = (NHO - wo_args[3]) // QH
                for h in range(QH):
                    pvs[h] = ps.tile([128, IB], F32, tag=f"pv{h % 2}",
                                     bufs=1, name=f"pv{h}")
                    if dve_sums:
                        accs[h] = pp.tile([128, IB], BF16, tag=f"acc{h % 2}",
                                          bufs=1, name=f"acc{h}")
                    else:
                        sms[h] = ps.tile([128, IB], F32, tag="ps",
                                         name=f"sm{h}")
                    for jt in range(njt):
                        kd = jt - ib * 4
                        f0 = kd * 128 if kd > 0 else 0   # diagonal trim
                        F = IB - f0
                        if kd == 3:
                            # all 4 heads' 128-wide kd3 tiles in one matmul
                            # + one exp: amortizes ACT's ~260ns fixed cost
                            if h == 0:
                                sc = ps.tile([128, IB], F32, tag=f"sc{jt % 2}",
                                             name="sc", bufs=1)
                                nc.tensor.matmul(sc[:],
                                                 kT[:, jt * 128:(jt + 1) * 128],
                                                 qT[:, 0:QH, i0 + f0:i0 + IB],
                                                 start=True, stop=True)
                                pbd = pp.tile([128, QH, 128], BF16,
                                              tag="pbd3", bufs=1, name="pbd3")
                                nc.scalar.activation(pbd[:], sc[:], EXP)
                                nc.vector.tensor_mul(pbd[:], pbd[:], mk4_s[:])
                                dstate[3] = pbd
                            ppb = dstate[3][:, h]
                        elif kd == 2:
                            # head pairs: 2x256 per matmul/exp
                            if h % 2 == 0:
                                sc = ps.tile([128, IB], F32, tag=f"sc{jt % 2}",
                                             name="sc", bufs=1)
                                nc.tensor.matmul(sc[:],
                                                 kT[:, jt * 128:(jt + 1) * 128],
                                                 qT[:, h:h + 2, i0 + f0:i0 + IB],
                                                 start=True, stop=True)
                                pbd = pp.tile([128, 2, 256], BF16,
                                              tag=f"pbd2{h // 2}", bufs=1,
                                              name="pbd2")
                                nc.scalar.activation(pbd[:], sc[:], EXP)
                                nc.vector.tensor_mul(pbd[:, :, 0:128],
                                                     pbd[:, :, 0:128],
                                                     mk4_s[:, 0:2])
                                dstate[2, h // 2] = pbd
                            ppb = dstate[2, h // 2][:, h % 2]
                        else:
                            sc = ps.tile([128, IB], F32, tag=f"sc{jt % 2}",
                                         name="sc", bufs=1)
                            nc.tensor.matmul(sc[:, :F],
                                             kT[:, jt * 128:(jt + 1) * 128],
                                             qT[:, h, i0 + f0:i0 + IB],
                                             start=True, stop=True)
                            pb = pp.tile([128, IB], BF16, tag=f"pb{jt % 5}",
                                         bufs=1, name="pb")
                            nc.scalar.activation(pb[:, :F], sc[:, :F], EXP)
                            if kd >= 0:
                                # mask the 128-wide diagonal strip
                                nc.vector.tensor_mul(pb[:, :128], pb[:, :128],
                                                     mk_s[:])
                            ppb = pb[:, :F]
                        if dve_sums:
                            if jt == 0:
                                nc.vector.tensor_copy(accs[h][:], ppb)
                            else:
                                nc.vector.tensor_add(accs[h][:, f0:],
                                                     accs[h][:, f0:], ppb)
                        pend.append((h, jt, ppb, f0))
                        if len(pend) > 4:
                            flush()
                    # head boundary: weave the head's leftover PV flushes
                    # through this head's wo chunk, then close the head
                    if wo_args is not None:
                        wb, wib, wat, ho0 = wo_args
                        lo = ho0 + h * per_head
                        hi = ho0 + (h + 1) * per_head
                        # ib==3 blocks have no ACT slack (exps saturate it) —
                        # except after the last head, where ACT helping the og
                        # drain frees PSUM slots for the next block's prefix
                        oge = "mix" if (ib < 3 or h == QH - 1) else "dve"
                        for ho in range(lo, hi):
                            emit_wo(wb, wib, wat, ho, ho + 1, oge)
                            if pend:
                                flush()
                    if dve_sums:
                        while pend:
                            flush()
                        finalize(h)
                # no-wo path keeps the cross-head pend pipeline; drain at end
                while pend:
                    flush()
                return at

            # ---- the pipeline over (sb, b) blocks ----
            # block n: xt(n) loads, wo(n-2) prefix, proj(n), rotary(n),
            # v-trans(n), attention(n-1) x wo(n-2) chunks
            qTs, kTs, vns = {}, {}, {}
            blocks = [(sb, b) for sb in range(NSB) for b in range(B)]
            att_q = []     # (b, ib, qT, kT, vn) awaiting attention
            wo_q = []      # (wb, wib, wat) awaiting wo emission
            xt_pre = {}    # next block's prefetched xt group tiles

            for n, (sb, b) in enumerate(blocks):
                if sb == 0:
                    qTs[b] = st.tile([128, QH, S], BF16, tag=f"qT{b}",
                                     name=f"qT{b}")
                    kTs[b] = st.tile([128, S], BF16, tag=f"kT{b}",
                                     name=f"kT{b}")
                    vns[b] = st.tile([128, NJT, HD], BF16, tag=f"vn{b}",
                                     name=f"vn{b}")
                qT, kT, vn = qTs[b], kTs[b], vns[b]
                s0 = sb * SB

                # x loads first: they land under the wo prefix + early proj
                # (hg 0/1 may already be in flight from last block's prefetch)
                xt_gs = []
                for hg in range(HC // 4):
                    if hg in xt_pre:
                        xt_gs.append(xt_pre.pop(hg))
                        continue
                    xt_g = xp.tile([128, 4, SB], BF16, tag=f"xt{hg % 4}",
                                   bufs=1, name=f"xt_g{hg}")
                    if n == 0:
                        # block 0 is feed-rate critical: 2-chunk quanta in
                        # strict consumption order, round-robin across the
                        # three queues, so delivery tracks the projection
                        _wchunk_loaded[hg] = True
                        for half in range(2):
                            hc0 = 4 * hg + 2 * half
                            eng = (nc.sync, nc.gpsimd, nc.scalar)[(2 * hg + half) % 3]
                            r0, r1 = hc0 * 128, (hc0 + 2) * 128
                            eng.dma_start(
                                out=wq_s[:, hc0:hc0 + 2],
                                in_=wq_e.ap()[r0:r1].rearrange("(c p) r -> p c r", c=2))
                            eng.dma_start(
                                out=wk_s[:, hc0:hc0 + 2],
                                in_=wk_e.ap()[r0:r1].rearrange("(c p) r -> p c r", c=2))
                            eng.dma_start(
                                out=wv_s[:, hc0:hc0 + 2],
                                in_=wv_e.ap()[r0:r1].rearrange("(c p) r -> p c r", c=2))
                            eng.dma_start(
                                out=xt_g[:, 2 * half:2 * half + 2],
                                in_=xt_e.ap()[0, r0:r1, 0:SB]
                                .rearrange("(c p) s -> p c s", c=2))
                    else:
                        load_wchunk(hg)
                        eng = nc.gpsimd if hg % 2 == 0 else nc.sync
                        eng.dma_start(
                            out=xt_g[:],
                            in_=xt_e.ap()[b, hg * 512:(hg + 1) * 512, s0:s0 + SB]
                            .rearrange("(c p) s -> p c s", c=4))
                    xt_gs.append(xt_g)
                if not _freqs_loaded[0]:
                    _freqs_loaded[0] = True
                    nc.scalar.dma_start(out=ccb[:], in_=cc_e.ap())
                    nc.scalar.dma_start(out=ssb[:], in_=ss_e.ap())
                    load_tail_weights()
                if n == 1:
                    load_wo()

                # wo prefix: covers this block's x DMA latency
                if wo_q:
                    wb_, wib_, wat_ = wo_q[0]
                    emit_wo(wb_, wib_, wat_, 0, WO_PRE, "act")



                # ---- projection + rotary for seq block sb ----
                qa = [ps.tile([128, SB], F32, tag="ps", name=f"qa{rc}")
                      for rc in range(QH)]
                ka = ps.tile([128, SB], F32, tag="sc0", name="ka", bufs=1)
                vta = ps.tile([128, SB], F32, tag="sc1", name="vta", bufs=1)
                for hg in range(HC // 4):
                    xt_g = xt_gs[hg]
                    for c in range(4):
                        hc = hg * 4 + c
                        st_, sp_ = (hc == 0), (hc == HC - 1)
                        for rc in range(QH):
                            nc.tensor.matmul(qa[rc][:],
                                             wq_s[:, hc, rc * 128:(rc + 1) * 128],
                                             xt_g[:, c], start=st_, stop=sp_)
                        nc.tensor.matmul(ka[:], wk_s[:, hc], xt_g[:, c],
                                         start=st_, stop=sp_)
                        nc.tensor.matmul(vta[:], wv_s[:, hc], xt_g[:, c],
                                         start=st_, stop=sp_)

                # stage 1: drain all projection PSUM banks to SBUF fast
                kq = rot_stage(ka, "qs_k", cp="act")
                vt_s = rp.tile([128, SB], BF16, tag="vts", bufs=2)
                nc.scalar.activation(vt_s[:], vta[:], COPY)
                qsl = [rot_stage(qa[rc], f"qs{rc}") for rc in range(QH)]
                # vT [d, s] -> natural v j-tiles via PE transpose (bf16)
                for t in range(SB // 128):
                    tp = ps.tile([128, 128], BF16, tag="ps", name="tp")
                    nc.tensor.transpose(tp[:], vt_s[:, t * 128:(t + 1) * 128],
                                        id_s[:])
                    nc.scalar.activation(vn[:, (SB // 128) * sb + t], tp[:], COPY)
                # stage 2: the rotary math (DVE), nothing downstream waits on
                # it until the next block's attention
                rot_math(kq, s0, kT[:, s0:s0 + SB])
                for rc in range(QH):
                    rot_math(qsl[rc], s0, qT[:, rc, s0:s0 + SB])

                # ---- attention(n-1) interleaved with wo(n-2) chunks ----
                if att_q:
                    ab, aib, aqT, akT, avn = att_q.pop(0)
                    if wo_q:
                        wb_, wib_, wat_ = wo_q.pop(0)
                        wargs = (wb_, wib_, wat_, WO_PRE)
                    else:
                        wargs = None
                    at = attention(ab, aib, aqT, akT, avn, wargs)
                    wo_q.append((ab, aib, at))
                att_q.append((b, sb, qT, kT, vn))

            # ---- tail: attention(7) x wo(6), then wo(7) ----
            ab, aib, aqT, akT, avn = att_q.pop(0)
            wb_, wib_, wat_ = wo_q.pop(0)
            emit_wo(wb_, wib_, wat_, 0, WO_PRE, "act")
            at = attention(ab, aib, aqT, akT, avn, (wb_, wib_, wat_, WO_PRE))
            wo_q.append((ab, aib, at))
            wb_, wib_, wat_ = wo_q.pop(0)
            emit_wo(wb_, wib_, wat_, 0, NHO, "mix", final=True)

    nc.compile()
    return nc


def _prep(x, freqs_cos, freqs_sin, wq, wk, wv, wo):
    """Shard + pre-transpose inputs for the 8 cores."""
    # rotary pair permutation: within each 32-partition quadrant, x0 of
    # pairs [16q,16q+16) sits in local slots 0..15 and x1 in 16..31.
    perm = np.zeros(HD, dtype=np.int64)
    pair = np.zeros(128, dtype=np.int64)
    sign = np.zeros(128, dtype=np.float32)
    for q in range(4):
        for t in range(16):
            perm[32 * q + t] = 2 * (16 * q + t)
            perm[32 * q + 16 + t] = 2 * (16 * q + t) + 1
            pair[32 * q + t] = 16 * q + t
            pair[32 * q + 16 + t] = 16 * q + t
            sign[32 * q + t] = -1.0
            sign[32 * q + 16 + t] = 1.0

    xt = np.ascontiguousarray(x.transpose(0, 2, 1)).astype(BF16NP)  # [B,H,S]
    # freqs identical across batches: ship [128, S] once, bf16
    cc = np.ascontiguousarray(freqs_cos[0][:, pair].T).astype(BF16NP)
    ss2 = np.ascontiguousarray((freqs_sin[0][:, pair] * sign[None, :]).T).astype(BF16NP)

    # lower-triangular mask for the 128-wide diagonal strip of each
    # trimmed diagonal j-tile
    jj = np.arange(128)[:, None]
    ii = np.arange(128)[None, :]
    maskd = (jj <= ii).astype(BF16NP)

    scale = np.float32(1.0 / np.sqrt(HD))
    in_maps = []
    for c in range(NCORES):
        wq_c = (wq[c * RQ:(c + 1) * RQ] * scale).reshape(QH, HD, H)[:, perm, :]
        wqt = np.ascontiguousarray(wq_c.reshape(RQ, H).T).astype(BF16NP)
        wk_c = wk[c * HD:(c + 1) * HD][perm, :]
        wkt = np.ascontiguousarray(wk_c.T).astype(BF16NP)
        wvt = np.ascontiguousarray(wv[c * HD:(c + 1) * HD].T).astype(BF16NP)
        wot = np.ascontiguousarray(wo[:, c * RQ:(c + 1) * RQ].T).astype(BF16NP)
        in_maps.append({
            "xt": xt, "cc": cc, "ss2": ss2,
            "wqt": wqt, "wkt": wkt, "wvt": wvt, "wot": wot,
            "maskd": maskd,
            "maskd4": np.ascontiguousarray(np.tile(maskd, (1, 4))),
            "ident": np.eye(128, dtype=BF16NP),
        })
    return in_maps


def kernel(x, freqs_cos, freqs_sin, wq, wk, wv, wo):
    global _CACHED, LAST_EXEC_NS, LAST_RES
    x = np.asarray(x, dtype=np.float32)
    freqs_cos = np.asarray(freqs_cos, dtype=np.float32)
    freqs_sin = np.asarray(freqs_sin, dtype=np.float32)
    wq = np.asarray(wq, dtype=np.float32)
    wk = np.asarray(wk, dtype=np.float32)
    wv = np.asarray(wv, dtype=np.float32)
    wo = np.asarray(wo, dtype=np.float32)

    if _CACHED is None:
        _CACHED = _build()
    nc = _CACHED

    in_maps = _prep(x, freqs_cos, freqs_sin, wq, wk, wv, wo)
    res = run_bass_kernel_spmd(nc, in_maps, core_ids=list(range(NCORES)))
    LAST_EXEC_NS = res.exec_time_ns
    LAST_RES = res

    # unshard: sum the 8 partial wo products, then [B,H,S] -> [B,S,H]
    acc = res.results[0]["out"].astype(np.float64)
    for c in range(1, NCORES):
        acc += res.results[c]["out"]
    return np.ascontiguousarray(acc.transpose(0, 2, 1)).astype(np.float32)


# revision 41
# speedup vs baseline: 1.2007x; 1.2007x over previous
"""Multi-head GQA attention (B=2, S=2048, H=4096, 32 q-heads / 8 kv-heads,
HD=128, rotary, causal) on 8 TRN2 NeuronCores.

Sharding: tensor-parallel over heads, 8-way — core c owns q-heads
[4c, 4c+4) and kv-head c; wq/wk/wv column-sharded, wo row-sharded.  Each
core computes a partial wo product over its head slice for both batches;
the host sums the 8 partials (the TP reduction) and transposes back.

All on-device dataflow is in transposed layout [feature, seq] so no
device-side transposes are needed; the host pre-transposes x and the
weight shards during sharding.  Rotary pairs are laid out so the (x0,x1)
pair swap is a 32-partition-quadrant stream_shuffle on the Vector engine.

v5 structural changes vs v2/v3:
- the attention phase is ACT-bound (exp is ~687ns/tile, not pipelined),
  so attention for block n-1 runs during block n, with the wo matmuls of
  block n-2 interleaved one 6-ho chunk after each head: the PE chews wo
  while ACT streams exps, and neither waits for the other.
- attention lagging a full block means rotary/kT/vn never gate scores
  (50us of slack), so the old half1/half2 rotary-coverage machinery is
  gone; a small 8-ho wo prefix before each projection covers the x DMA.
- softmax denominators accumulate on DVE in bf16 (tile adds); one
  ones-matmul per head at its chunk end replaces the per-tile sums
  matmul (PE pays 512 cols/head instead of a full extra sums pass).
- PSUM is re-tagged: a 4-deep rotation (qa x4 / tp / wo accs / sm),
  ka/vta borrow the two score banks (disjoint phases), and pvs get two
  dedicated banks - projection still sees 6 live banks, attention gets
  a collision-free pipeline.
- rotary runs fully on DVE in bf16 (freqs shipped bf16): ~2x cheaper
  and off ACT's critical path.
"""
import sys

if "/opt/trn_rl_repo" not in sys.path:
    sys.path.insert(0, "/opt/trn_rl_repo")

import numpy as np
import ml_dtypes

from concourse import bacc, tile, mybir
from concourse.bass_utils import run_bass_kernel_spmd

F32 = mybir.dt.float32
BF16 = mybir.dt.bfloat16
EXP = mybir.ActivationFunctionType.Exp
COPY = mybir.ActivationFunctionType.Copy
BF16NP = ml_dtypes.bfloat16

B, S, H = 2, 2048, 4096
NH, NKV, HD = 32, 8, 128
NCORES = 8
QH = NH // NCORES          # 4 q-heads per core
RQ = QH * HD               # 512 q rows per core
SB = 512                   # projection seq block
NSB = S // SB              # 4
IB = 512                   # attention i block
NIB = S // IB              # 4
HC = H // 128              # 32 contraction chunks
NJT = S // 128             # 16 j tiles
NHO = H // 128             # 32 wo output row-chunks
WO_PRE = 4                 # wo chunks emitted before the projection

# stream_shuffle mask: swap 16-partition halves within each 32-partition quadrant
SHUF = list(range(16, 32)) + list(range(16))

LAST_EXEC_NS = None
LAST_RES = None
_CACHED = None


def _build():
    nc = bacc.Bacc("TRN2", target_bir_lowering=False, debug=False,
                   num_devices=NCORES)

    xt_e = nc.dram_tensor("xt", [B, H, S], BF16, kind="ExternalInput")
    cc_e = nc.dram_tensor("cc", [128, S], BF16, kind="ExternalInput")
    ss_e = nc.dram_tensor("ss2", [128, S], BF16, kind="ExternalInput")
    wq_e = nc.dram_tensor("wqt", [H, RQ], BF16, kind="ExternalInput")
    wk_e = nc.dram_tensor("wkt", [H, HD], BF16, kind="ExternalInput")
    wv_e = nc.dram_tensor("wvt", [H, HD], BF16, kind="ExternalInput")
    wo_e = nc.dram_tensor("wot", [RQ, H], BF16, kind="ExternalInput")
    mk_e = nc.dram_tensor("maskd", [128, 128], BF16, kind="ExternalInput")
    mk4_e = nc.dram_tensor("maskd4", [128, 512], BF16, kind="ExternalInput")
    id_e = nc.dram_tensor("ident", [128, 128], BF16, kind="ExternalInput")
    out_e = nc.dram_tensor("out", [B, H, S], BF16, kind="ExternalOutput")

    with tile.TileContext(nc) as tc:
        with (nc.allow_low_precision(reason="bf16 compute by design"),
              tc.tile_pool(name="wpool", bufs=1) as wp,
              tc.tile_pool(name="state", bufs=1) as st,
              tc.tile_pool(name="att", bufs=3) as ap_,
              tc.tile_pool(name="xin", bufs=4) as xp,
              tc.tile_pool(name="probs", bufs=1) as pp,
              tc.tile_pool(name="rot", bufs=1) as rp,
              tc.tile_pool(name="stage", bufs=1) as sg,
              tc.tile_pool(name="ps", bufs=4, space="PSUM") as ps):

            # ---- resident weights ----
            wq_s = wp.tile([128, HC, RQ], BF16)
            wk_s = wp.tile([128, HC, HD], BF16)
            wv_s = wp.tile([128, HC, HD], BF16)
            wo_s = wp.tile([128, QH, H], BF16)
            mk_s = wp.tile([128, 128], BF16)
            mk4_s = wp.tile([128, QH, 128], BF16)
            ones128 = wp.tile([128, 128], BF16)
            id_s = wp.tile([128, 128], BF16)
            nc.vector.memset(ones128[:], 1.0)

            # PE warm-up: harmless matmuls that run during the initial DMA
            # wait; they pull the PE out of its low p-state and cover the
            # first x/weight transfers
            warm = wp.tile([128, 512], BF16)
            nc.vector.memset(warm[:], 0.0)
            wps = ps.tile([128, 512], F32, tag="ps", name="wps")
            for i in range(12):
                nc.tensor.matmul(wps[:], ones128[:], warm[:],
                                 start=(i == 0), stop=(i == 11))

            _wchunk_loaded = [False] * (HC // 4)
            _tail_loaded = [False]
            _wo_loaded = [False]

            def load_wchunk(hg):
                # one grouped DMA per 4 contraction chunks.  wq alternates
                # sync/gpsimd, wk+wv ride the otherwise-idle scalar queue, and
                # xt uses the opposite parity: each queue then delivers in
                # consumption order and none serializes the startup.
                if _wchunk_loaded[hg]:
                    return
                _wchunk_loaded[hg] = True
                eng = nc.sync if hg % 2 == 0 else nc.gpsimd
                r0, r1 = hg * 512, (hg + 1) * 512
                eng.dma_start(
                    out=wq_s[:, 4 * hg:4 * hg + 4],
                    in_=wq_e.ap()[r0:r1].rearrange("(c p) r -> p c r", c=4))
                nc.scalar.dma_start(
                    out=wk_s[:, 4 * hg:4 * hg + 4],
                    in_=wk_e.ap()[r0:r1].rearrange("(c p) r -> p c r", c=4))
                nc.scalar.dma_start(
                    out=wv_s[:, 4 * hg:4 * hg + 4],
                    in_=wv_e.ap()[r0:r1].rearrange("(c p) r -> p c r", c=4))

            def load_tail_weights():
                if _tail_loaded[0]:
                    return
                _tail_loaded[0] = True
                nc.gpsimd.dma_start(out=id_s[:], in_=id_e.ap())
                nc.gpsimd.dma_start(out=mk_s[:], in_=mk_e.ap())
                nc.gpsimd.dma_start(out=mk4_s[:], in_=mk4_e.ap())

            def load_wo():
                # wo (4MB) is first read during block 2; loading it in block
                # 0 oversubscribes the startup DMA window, so it goes out on
                # the idle scalar queue at block 1
                if _wo_loaded[0]:
                    return
                _wo_loaded[0] = True
                for rc in range(QH):
                    nc.scalar.dma_start(out=wo_s[:, rc],
                                        in_=wo_e.ap()[rc * 128:(rc + 1) * 128])

            # ---- wo emission: one at-tile is consumed as 32 ho chunks ----
            _og_n = [0]

            def emit_wo(wb, wib, wat, ho_lo, ho_hi, og_eng, final=False):
                wi0 = wib * IB
                for ho in range(ho_lo, ho_hi):
                    acc = ps.tile([128, IB], F32, tag="ps", name="acc")
                    for rc in range(QH):
                        nc.tensor.matmul(acc[:],
                                         wo_s[:, rc, ho * 128:(ho + 1) * 128],
                                         wat[:, rc],
                                         start=(rc == 0), stop=(rc == QH - 1))
                    g = _og_n[0] = (_og_n[0] + 1) % 8
                    og = sg.tile([128, IB], BF16, tag=f"og{g}", name="og",
                                 bufs=1)
                    if og_eng == "act" or (og_eng == "mix" and ho % 2 == 0):
                        nc.scalar.activation(og[:], acc[:], COPY)
                    else:
                        nc.vector.tensor_copy(og[:], acc[:])
                    if final:
                        oeng = (nc.gpsimd, nc.sync, nc.scalar)[ho % 3]
                    else:
                        oeng = nc.gpsimd if ho % 2 == 0 else nc.sync
                    oeng.dma_start(
                        out=out_e.ap()[wb, ho * 128:(ho + 1) * 128, wi0:wi0 + IB],
                        in_=og[:])

            # freqs are identical for both batches: load once (bf16)
            ccb = st.tile([128, S], BF16, tag="cc")
            ssb = st.tile([128, S], BF16, tag="ss")
            _freqs_loaded = [False]

            _rot_n = [0]

            def rot_stage(src_ps, tag, cp="dve"):
                # PSUM -> SBUF bf16 staging.  Staging all five rotary inputs
                # FIRST releases their PSUM banks quickly: the v-transpose
                # reuses qa0's slot and the next attention's first scores
                # matmul reuses ka's (sc0), so these reads gate the PE.
                qs = rp.tile([128, SB], BF16, tag=tag, bufs=1)
                if cp == "act":
                    nc.scalar.activation(qs[:], src_ps[:], COPY)
                else:
                    nc.vector.tensor_copy(qs[:], src_ps[:])
                return qs

            def rot_math(qs, s0, dst_ap):
                # dst = qs*CC + shuffle(qs)*SS2, bf16 on DVE
                r = _rot_n[0] = (_rot_n[0] + 1) % 2
                qw = rp.tile([128, SB], BF16, tag=f"qw{r}", bufs=1)
                nc.vector.stream_shuffle(qw[:], qs[:], SHUF)
                nc.vector.tensor_mul(qs[:], qs[:], ccb[:, s0:s0 + SB])
                nc.vector.tensor_mul(qw[:], qw[:], ssb[:, s0:s0 + SB])
                nc.vector.tensor_add(dst_ap, qs[:], qw[:])

            # ---- attention for one (b, ib), interleaved with wo chunks ----
            def attention(b, ib, qT, kT, vn, wo_args):
                # wo_args: (wb, wib, wat) of the 2-blocks-ago at, or None.
                # Emits a 6-ho wo chunk after each head so the PE stays busy
                # while ACT streams the next head's exps; the chunk also
                # gives the DVE time to finish the head's prob-sum adds
                # before the ones-matmul (finalize) needs them.
                i0 = ib * IB
                njt = (ib + 1) * (IB // 128)
                at = ap_.tile([128, QH, IB], BF16, tag="at", name="at")
                pvs = [None] * QH
                sms = [None] * QH
                accs = [None] * QH
                # without wo filler (block 1 only) the chunk trick can't
                # hide the DVE lag: keep per-tile PE sums there
                dve_sums = wo_args is not None
                pend = []
                dstate = {}    # merged diagonal-tile probs, shared by heads

                def flush():
                    ph, pjt, ppb, pf0 = pend.pop(0)
                    st_ = (pjt == 0)
                    sp_ = (pjt == njt - 1)
                    nc.tensor.matmul(pvs[ph][:, pf0:], vn[:, pjt], ppb,
                                     start=st_, stop=sp_)
                    if not dve_sums:
                        nc.tensor.matmul(sms[ph][:, pf0:], ones128[:], ppb,
                                         start=st_, stop=sp_)
                        if sp_:
                            rb = sg.tile([128, IB], F32, tag=f"rb{ph % 2}",
                                         bufs=1, name="rb")
                            nc.vector.reciprocal_approx_fast(rb[:], sms[ph][:])
                            nc.vector.tensor_mul(at[:, ph], pvs[ph][:], rb[:])

                def finalize(ph):
                    sm = ps.tile([128, IB], F32, tag="ps", name=f"sm{ph}")
                    nc.tensor.matmul(sm[:], ones128[:], accs[ph][:],
                                     start=True, stop=True)
                    rb = sg.tile([128, IB], F32, tag=f"rb{ph % 2}",
                                 bufs=1, name="rb")
                    nc.vector.reciprocal_approx_fast(rb[:], sm[:])
                    nc.vector.tensor_mul(at[:, ph], pvs[ph][:], rb[:])

                # wo chunk schedule: 6 ho per head when in the steady loop
                # (prefix of 8 emitted before the projection), 8 per head in
                # the tail where there is no prefix
                if wo_args is not None:
                    per_head = (NHO - wo_args[3]) // QH
                for h in range(QH):
                    pvs[h] = ps.tile([128, IB], F32, tag=f"pv{h % 2}",
                                     bufs=1, name=f"pv{h}")
                    if dve_sums:
                        accs[h] = pp.tile([128, IB], BF16, tag=f"acc{h % 2}",
                                          bufs=1, name=f"acc{h}")
                    else:
                        sms[h] = ps.tile([128, IB], F32, tag="ps",
                                         name=f"sm{h}")
                    for jt in range(njt):
                        kd = jt - ib * 4
                        f0 = kd * 128 if kd > 0 else 0   # diagonal trim
                        F = IB - f0
                        sc = ps.tile([128, IB], F32, tag=f"sc{jt % 2}",
                                     name="sc", bufs=1)
                        nc.tensor.matmul(sc[:, :F],
                                         kT[:, jt * 128:(jt + 1) * 128],
                                         qT[:, h, i0 + f0:i0 + IB],
                                         start=True, stop=True)
                        pb = pp.tile([128, IB], BF16, tag=f"pb{jt % 5}",
                                     bufs=1, name="pb")
                        nc.scalar.activation(pb[:, :F], sc[:, :F], EXP)
                        if kd >= 0:
                            # mask the 128-wide diagonal strip
                            nc.vector.tensor_mul(pb[:, :128], pb[:, :128],
                                                 mk_s[:])
                        ppb = pb[:, :F]
                        if dve_sums:
                            if jt == 0:
                                nc.vector.tensor_copy(accs[h][:], ppb)
                            else:
                                nc.vector.tensor_add(accs[h][:, f0:],
                                                     accs[h][:, f0:], ppb)
                        pend.append((h, jt, ppb, f0))
                        if len(pend) > 4:
                            flush()
                    # head boundary: weave the head's leftover PV flushes
                    # through this head's wo chunk, then close the head
                    if wo_args is not None:
                        wb, wib, wat, ho0 = wo_args
                        lo = ho0 + h * per_head
                        hi = ho0 + (h + 1) * per_head
                        # ib==3 blocks have no ACT slack (exps saturate it) —
                        # except after the last head, where ACT helping the og
                        # drain frees PSUM slots for the next block's prefix
                        oge = "mix" if (ib < 3 or h == QH - 1) else "dve"
                        for ho in range(lo, hi):
                            emit_wo(wb, wib, wat, ho, ho + 1, oge)
                            if pend:
                                flush()
                    if dve_sums:
                        while pend:
                            flush()
                        finalize(h)
                # no-wo path keeps the cross-head pend pipeline; drain at end
                while pend:
                    flush()
                return at

            # ---- the pipeline over (sb, b) blocks ----
            # block n: xt(n) loads, wo(n-2) prefix, proj(n), rotary(n),
            # v-trans(n), attention(n-1) x wo(n-2) chunks
            qTs, kTs, vns = {}, {}, {}
            blocks = [(sb, b) for sb in range(NSB) for b in range(B)]
            att_q = []     # (b, ib, qT, kT, vn) awaiting attention
            wo_q = []      # (wb, wib, wat) awaiting wo emission
            xt_pre = {}    # next block's prefetched xt group tiles

            for n, (sb, b) in enumerate(blocks):
                if sb == 0:
                    qTs[b] = st.tile([128, QH, S], BF16, tag=f"qT{b}",
                                     name=f"qT{b}")
                    kTs[b] = st.tile([128, S], BF16, tag=f"kT{b}",
                                     name=f"kT{b}")
                    vns[b] = st.tile([128, NJT, HD], BF16, tag=f"vn{b}",
                                     name=f"vn{b}")
                qT, kT, vn = qTs[b], kTs[b], vns[b]
                s0 = sb * SB

                # x loads first: they land under the wo prefix + early proj
                # (hg 0/1 may already be in flight from last block's prefetch)
                xt_gs = []
                for hg in range(HC // 4):
                    if hg in xt_pre:
                        xt_gs.append(xt_pre.pop(hg))
                        continue
                    xt_g = xp.tile([128, 4, SB], BF16, tag=f"xt{hg % 4}",
                                   bufs=1, name=f"xt_g{hg}")
                    if n == 0:
                        # block 0 is feed-rate critical: 2-chunk quanta in
                        # strict consumption order, round-robin across the
                        # three queues, so delivery tracks the projection
                        _wchunk_loaded[hg] = True
                        for half in range(2):
                            hc0 = 4 * hg + 2 * half
                            eng = (nc.sync, nc.gpsimd, nc.scalar)[(2 * hg + half) % 3]
                            r0, r1 = hc0 * 128, (hc0 + 2) * 128
                            eng.dma_start(
                                out=wq_s[:, hc0:hc0 + 2],
                                in_=wq_e.ap()[r0:r1].rearrange("(c p) r -> p c r", c=2))
                            eng.dma_start(
                                out=wk_s[:, hc0:hc0 + 2],
                                in_=wk_e.ap()[r0:r1].rearrange("(c p) r -> p c r", c=2))
                            eng.dma_start(
                                out=wv_s[:, hc0:hc0 + 2],
                                in_=wv_e.ap()[r0:r1].rearrange("(c p) r -> p c r", c=2))
                            eng.dma_start(
                                out=xt_g[:, 2 * half:2 * half + 2],
                                in_=xt_e.ap()[0, r0:r1, 0:SB]
                                .rearrange("(c p) s -> p c s", c=2))
                    else:
                        load_wchunk(hg)
                        eng = nc.gpsimd if hg % 2 == 0 else nc.sync
                        eng.dma_start(
                            out=xt_g[:],
                            in_=xt_e.ap()[b, hg * 512:(hg + 1) * 512, s0:s0 + SB]
                            .rearrange("(c p) s -> p c s", c=4))
                    xt_gs.append(xt_g)
                if not _freqs_loaded[0]:
                    _freqs_loaded[0] = True
                    nc.scalar.dma_start(out=ccb[:], in_=cc_e.ap())
                    nc.scalar.dma_start(out=ssb[:], in_=ss_e.ap())
                    load_tail_weights()
                if n == 1:
                    load_wo()

                # wo prefix: covers this block's x DMA latency
                if wo_q:
                    wb_, wib_, wat_ = wo_q[0]
                    emit_wo(wb_, wib_, wat_, 0, WO_PRE, "act")



                # ---- projection + rotary for seq block sb ----
                qa = [ps.tile([128, SB], F32, tag="ps", name=f"qa{rc}")
                      for rc in range(QH)]
                ka = ps.tile([128, SB], F32, tag="sc0", name="ka", bufs=1)
                vta = ps.tile([128, SB], F32, tag="sc1", name="vta", bufs=1)
                for hg in range(HC // 4):
                    xt_g = xt_gs[hg]
                    for c in range(4):
                        hc = hg * 4 + c
                        st_, sp_ = (hc == 0), (hc == HC - 1)
                        for rc in range(QH):
                            nc.tensor.matmul(qa[rc][:],
                                             wq_s[:, hc, rc * 128:(rc + 1) * 128],
                                             xt_g[:, c], start=st_, stop=sp_)
                        nc.tensor.matmul(ka[:], wk_s[:, hc], xt_g[:, c],
                                         start=st_, stop=sp_)
                        nc.tensor.matmul(vta[:], wv_s[:, hc], xt_g[:, c],
                                         start=st_, stop=sp_)

                # stage 1: drain all projection PSUM banks to SBUF fast
                kq = rot_stage(ka, "qs_k", cp="act")
                vt_s = rp.tile([128, SB], BF16, tag="vts", bufs=2)
                nc.scalar.activation(vt_s[:], vta[:], COPY)
                qsl = [rot_stage(qa[rc], f"qs{rc}") for rc in range(QH)]
                # vT [d, s] -> natural v j-tiles via PE transpose (bf16)
                for t in range(SB // 128):
                    tp = ps.tile([128, 128], BF16, tag="ps", name="tp")
                    nc.tensor.transpose(tp[:], vt_s[:, t * 128:(t + 1) * 128],
                                        id_s[:])
                    nc.scalar.activation(vn[:, (SB // 128) * sb + t], tp[:], COPY)
                # stage 2: the rotary math (DVE), nothing downstream waits on
                # it until the next block's attention
                rot_math(kq, s0, kT[:, s0:s0 + SB])
                for rc in range(QH):
                    rot_math(qsl[rc], s0, qT[:, rc, s0:s0 + SB])

                # ---- attention(n-1) interleaved with wo(n-2) chunks ----
                if att_q:
                    ab, aib, aqT, akT, avn = att_q.pop(0)
                    if wo_q:
                        wb_, wib_, wat_ = wo_q.pop(0)
                        wargs = (wb_, wib_, wat_, WO_PRE)
                    else:
                        wargs = None
                    at = attention(ab, aib, aqT, akT, avn, wargs)
                    wo_q.append((ab, aib, at))
                att_q.append((b, sb, qT, kT, vn))

            # ---- tail: attention(7) x wo(6), then wo(7) ----
            ab, aib, aqT, akT, avn = att_q.pop(0)
            wb_, wib_, wat_ = wo_q.pop(0)
            emit_wo(wb_, wib_, wat_, 0, WO_PRE, "act")
            at = attention(ab, aib, aqT, akT, avn, (wb_, wib_, wat_, WO_PRE))
            wo_q.append((ab, aib, at))
            wb_, wib_, wat_ = wo_q.pop(0)
            emit_wo(wb_, wib_, wat_, 0, NHO, "mix", final=True)

    nc.compile()
    return nc


def _prep(x, freqs_cos, freqs_sin, wq, wk, wv, wo):
    """Shard + pre-transpose inputs for the 8 cores."""
    # rotary pair permutation: within each 32-partition quadrant, x0 of
    # pairs [16q,16q+16) sits in local slots 0..15 and x1 in 16..31.
    perm = np.zeros(HD, dtype=np.int64)
    pair = np.zeros(128, dtype=np.int64)
    sign = np.zeros(128, dtype=np.float32)
    for q in range(4):
        for t in range(16):
            perm[32 * q + t] = 2 * (16 * q + t)
            perm[32 * q + 16 + t] = 2 * (16 * q + t) + 1
            pair[32 * q + t] = 16 * q + t
            pair[32 * q + 16 + t] = 16 * q + t
            sign[32 * q + t] = -1.0
            sign[32 * q + 16 + t] = 1.0

    xt = np.ascontiguousarray(x.transpose(0, 2, 1)).astype(BF16NP)  # [B,H,S]
    # freqs identical across batches: ship [128, S] once, bf16
    cc = np.ascontiguousarray(freqs_cos[0][:, pair].T).astype(BF16NP)
    ss2 = np.ascontiguousarray((freqs_sin[0][:, pair] * sign[None, :]).T).astype(BF16NP)

    # lower-triangular mask for the 128-wide diagonal strip of each
    # trimmed diagonal j-tile
    jj = np.arange(128)[:, None]
    ii = np.arange(128)[None, :]
    maskd = (jj <= ii).astype(BF16NP)

    scale = np.float32(1.0 / np.sqrt(HD))
    in_maps = []
    for c in range(NCORES):
        wq_c = (wq[c * RQ:(c + 1) * RQ] * scale).reshape(QH, HD, H)[:, perm, :]
        wqt = np.ascontiguousarray(wq_c.reshape(RQ, H).T).astype(BF16NP)
        wk_c = wk[c * HD:(c + 1) * HD][perm, :]
        wkt = np.ascontiguousarray(wk_c.T).astype(BF16NP)
        wvt = np.ascontiguousarray(wv[c * HD:(c + 1) * HD].T).astype(BF16NP)
        wot = np.ascontiguousarray(wo[:, c * RQ:(c + 1) * RQ].T).astype(BF16NP)
        in_maps.append({
            "xt": xt, "cc": cc, "ss2": ss2,
            "wqt": wqt, "wkt": wkt, "wvt": wvt, "wot": wot,
            "maskd": maskd,
            "maskd4": np.ascontiguousarray(np.tile(maskd, (1, 4))),
            "ident": np.eye(128, dtype=BF16NP),
        })
    return in_maps


def kernel(x, freqs_cos, freqs_sin, wq, wk, wv, wo):
    global _CACHED, LAST_EXEC_NS, LAST_RES
    x = np.asarray(x, dtype=np.float32)
    freqs_cos = np.asarray(freqs_cos, dtype=np.float32)
    freqs_sin = np.asarray(freqs_sin, dtype=np.float32)
    wq = np.asarray(wq, dtype=np.float32)
    wk = np.asarray(wk, dtype=np.float32)
    wv = np.asarray(wv, dtype=np.float32)
    wo = np.asarray(wo, dtype=np.float32)

    if _CACHED is None:
        _CACHED = _build()
    nc = _CACHED

    in_maps = _prep(x, freqs_cos, freqs_sin, wq, wk, wv, wo)
    res = run_bass_kernel_spmd(nc, in_maps, core_ids=list(range(NCORES)))
    LAST_EXEC_NS = res.exec_time_ns
    LAST_RES = res

    # unshard: sum the 8 partial wo products, then [B,H,S] -> [B,S,H]
    acc = res.results[0]["out"].astype(np.float64)
    for c in range(1, NCORES):
        acc += res.results[c]["out"]
    return np.ascontiguousarray(acc.transpose(0, 2, 1)).astype(np.float32)
